# revision 20
# baseline (speedup 1.0000x reference)
"""Masked dot-product attention on 8 Trainium2 NeuronCores (Bass/Tile).

Problem: queries/keys/values [32, 1024, 128] f32, valid_lens [32] i32.
  out = softmax(mask(Q K^T / sqrt(128))) V        (key-padding prefix mask)

Strategy (batch-parallel, 4 batches per core, one SPMD program):
  * Host pre-transposes Q and K per batch to [D=128, 1024] so the
    contraction dim D sits on SBUF partitions; no on-device transposes.
  * Scores are computed transposed: S^T[k, q] = (K^T chunk).T @ Q^T with k
    in chunks of 128 partitions.
  * The prefix key mask is per-PARTITION in this layout, so it folds into
    the exp for free: ACT computes exp(S^T * 1/sqrt(D) + bias) with
    bias[k] in {0, -1e6}; masked rows become exactly 0.
  * out^T[v, q] += V_chunk-as-lhsT @ expS^T accumulates in PSUM across
    k chunks (V is loaded chunk-major, no transpose needed).
  * denominator: expS^T chunks are accumulated in SBUF on DVE (fp32),
    then one fp32 ones-column matmul per batch reduces across partitions.
  * out^T and sums are DMA'd back; the host divides and transposes
    while gathering (0.003% of the FLOPs).
  * float32r everywhere on the PE: 1 cycle/row instead of fp32's 4.

Static masked-chunk skipping: batch b only needs ceil(valid_lens[b]/128)
key chunks; the rest contribute exactly 0. Batches are assigned to the 4
per-core slots by descending need (sorted, slot-major), so slot j's
compile-time chunk count is max over its 8 batches. The SPMD program is
specialized to that profile at kernel build time.

The chunk loop is software-pipelined: chunk c+1's score matmuls are
emitted before chunk c's AV/sums matmuls so the PE produces the next
exp's input first and ACT never starves.
"""

import math
import os as _os

import numpy as np

import concourse.bacc as bacc
import concourse.bass as bass
import concourse.mybir as mybir
import concourse.tile as tile
from concourse.bass_utils import run_bass_kernel_spmd

B, Q, K, D = 32, 1024, 1024, 128
N_CORES = 8
BPC = B // N_CORES  # batches per core
PART = 128          # partition size / key chunk size
NCHUNK = K // PART
MASK_BIAS = -1.0e6
INV_SQRT_D = 1.0 / math.sqrt(D)
F32 = mybir.dt.float32
F32R = mybir.dt.float32r

_NC_CACHE: dict = {}


def build_nc(profile: tuple) -> bass.Bass:
    """Build the SPMD Bass program for a per-slot chunk-count profile."""
    nc = bacc.Bacc()
    qt = nc.declare_dram_parameter("qt", [BPC, PART, Q], F32R, isOutput=False)
    kt = nc.declare_dram_parameter("kt", [BPC, PART, K], F32R, isOutput=False)
    vp = nc.declare_dram_parameter("vp", [BPC, PART, K], F32R, isOutput=False)
    mb = nc.declare_dram_parameter("mb", [PART, BPC * NCHUNK], F32, isOutput=False)
    cst = nc.declare_dram_parameter("cst", [PART, 1], F32R, isOutput=False)
    out = nc.declare_dram_parameter("out", [BPC, PART, Q], F32, isOutput=True)
    sums_out = nc.declare_dram_parameter("sums", [BPC, 1, Q], F32, isOutput=True)

    with tile.TileContext(nc) as tc:
        with (
            tc.tile_pool(name="io", bufs=2) as io,
            tc.tile_pool(name="probs", bufs=8) as probs,
            tc.tile_pool(name="consts", bufs=1) as consts,
            tc.tile_pool(name="acc", bufs=2) as accp,
            tc.tile_pool(name="ps_s", bufs=2, space="PSUM") as ps_s,
            tc.tile_pool(name="ps_acc", bufs=2, space="PSUM") as ps_acc,
        ):
            # Startup-ordered loads: batch 0's operands first (SP HWDGE ring
            # is FIFO), then the small consts, then the rest.
            ins_sb = []
            for b in range(BPC):
                cap = profile[b]
                kcols = cap * PART
                qt_sb = io.tile([PART, Q], F32R, tag="qt", name=f"qt{b}")
                kt_sb = io.tile([PART, kcols], F32R, tag="kt", name=f"kt{b}")
                vp_sb = io.tile([PART, kcols], F32R, tag="vp", name=f"vp{b}")
                ins_sb.append((qt_sb, kt_sb, vp_sb))
                nc.sync.dma_start(out=kt_sb, in_=kt[b][:, :kcols])
                nc.sync.dma_start(out=qt_sb, in_=qt[b])
                if b == 0:
                    ones_col = consts.tile([PART, 1], F32R)
                    nc.sync.dma_start(out=ones_col, in_=cst[:, :])
                    mb_sb = consts.tile([PART, BPC * NCHUNK], F32)
                    nc.sync.dma_start(out=mb_sb, in_=mb[:, :])
                nc.sync.dma_start(out=vp_sb, in_=vp[b][:, :kcols])

            for b in range(BPC):
                cap = profile[b]
                qt_sb, kt_sb, vp_sb = ins_sb[b]

                out_ps = ps_acc.tile([PART, Q], F32, tag="out")
                acc_sb = accp.tile([PART, Q], F32, tag="acc")

                def s_mms(c):
                    s_ps = ps_s.tile([PART, Q], F32, tag="s", name=f"s_b{b}c{c}")
                    kw = kt_sb[:, c * PART:(c + 1) * PART]
                    for h in range(2):
                        nc.tensor.matmul(
                            s_ps[:, h * 512:(h + 1) * 512],
                            kw,
                            qt_sb[:, h * 512:(h + 1) * 512],
                            start=True,
                            stop=True,
                        )
                    return s_ps

                s_cur = s_mms(0)
                for c in range(cap):
                    p_sb = probs.tile([PART, Q], F32R, tag="p")
                    nc.scalar.activation(
                        p_sb,
                        s_cur,
                        mybir.ActivationFunctionType.Exp,
                        bias=mb_sb[:, b * NCHUNK + c:b * NCHUNK + c + 1],
                        scale=INV_SQRT_D,
                    )
                    if c + 1 < cap:
                        s_cur = s_mms(c + 1)
                    vw = vp_sb[:, c * PART:(c + 1) * PART]
                    first, last = c == 0, c == cap - 1
                    for h in range(2):
                        nc.tensor.matmul(
                            out_ps[:, h * 512:(h + 1) * 512],
                            vw,
                            p_sb[:, h * 512:(h + 1) * 512],
                            start=first,
                            stop=last,
                        )
                    if first:
                        nc.vector.tensor_copy(acc_sb, p_sb.bitcast(F32))
                    else:
                        nc.vector.tensor_add(acc_sb, acc_sb, p_sb.bitcast(F32))

                # Epilogue: fp32 cross-partition reduce of acc (exact; the
                # ones multiply is by 1.0), then PSUM -> SBUF -> DRAM.
                sums_ps = ps_s.tile([1, Q], F32, tag="s", name=f"sums_b{b}")
                for h in range(2):
                    nc.tensor.matmul(
                        sums_ps[:, h * 512:(h + 1) * 512],
                        ones_col.bitcast(F32),
                        acc_sb[:, h * 512:(h + 1) * 512],
                        start=True,
                        stop=True,
                    )
                outn = io.tile([PART, Q], F32, tag="outn")
                nc.scalar.copy(outn[:, 0:512], out_ps[:, 0:512])
                nc.vector.tensor_copy(outn[:, 512:1024], out_ps[:, 512:1024])
                nc.sync.dma_start(out=out[b], in_=outn)
                sums_sb = probs.tile([1, Q], F32, tag="sums_sb")
                nc.vector.tensor_copy(sums_sb, sums_ps)
                nc.sync.dma_start(out=sums_out[b], in_=sums_sb)

    nc.compile()
    return nc


def plan(valid_lens: np.ndarray):
    """Assign batches to (core, slot) and derive the chunk-count profile.

    Sorting by descending need and slicing slot-major minimizes the sum of
    per-slot maxima, which is the per-core static work.
    """
    need = np.minimum((valid_lens.astype(np.int64) + PART - 1) // PART, NCHUNK)
    need = np.maximum(need, 1)
    order = np.argsort(-need, kind="stable")
    perm = order.reshape(BPC, N_CORES)  # perm[slot, core] = batch index
    profile = tuple(int(need[perm[s]].max()) for s in range(BPC))
    return perm, profile


def kernel(queries, keys, values, valid_lens):
    q = np.ascontiguousarray(np.asarray(queries, dtype=np.float32))
    k = np.ascontiguousarray(np.asarray(keys, dtype=np.float32))
    v = np.ascontiguousarray(np.asarray(values, dtype=np.float32))
    lens = np.asarray(valid_lens).astype(np.int64).reshape(B)

    perm, profile = plan(lens)

    if profile not in _NC_CACHE:
        _NC_CACHE[profile] = build_nc(profile)
    nc = _NC_CACHE[profile]

    # Vectorized host layout prep: obi[core, slot] = batch index.
    obi = perm.T  # [N_CORES, BPC]
    qt_all = np.ascontiguousarray(q[obi].transpose(0, 1, 3, 2))  # [8,4,128,1024]
    kt_all = np.ascontiguousarray(k[obi].transpose(0, 1, 3, 2))
    # v chunk-major: vp[p, c*128 + d] = v[c*128 + p, d]
    vp_all = np.ascontiguousarray(
        v[obi]
        .reshape(N_CORES, BPC, NCHUNK, PART, D)
        .transpose(0, 1, 3, 2, 4)
        .reshape(N_CORES, BPC, PART, K)
    )
    # bias[p, slot*8 + c] = 0 if (c*128+p) < L else -1e6
    valid = np.arange(K)[None, None, :] < lens[obi][:, :, None]  # [8,4,1024]
    mb_all = np.where(
        valid.reshape(N_CORES, BPC, NCHUNK, PART).transpose(0, 2, 3, 1), 0.0, MASK_BIAS
    ).astype(np.float32)  # [8, NCHUNK, PART, BPC] -> need [8, PART, BPC*NCHUNK]
    mb_all = np.ascontiguousarray(
        mb_all.transpose(0, 2, 3, 1).reshape(N_CORES, PART, BPC * NCHUNK)
    )
    ones = np.ones((PART, 1), np.float32)

    in_maps = [
        {
            "qt": qt_all[core],
            "kt": kt_all[core],
            "vp": vp_all[core],
            "mb": mb_all[core],
            "cst": ones,
        }
        for core in range(N_CORES)
    ]

    res = run_bass_kernel_spmd(nc, in_maps, list(range(N_CORES)))

    out = np.empty((B, Q, D), np.float32)
    for core in range(N_CORES):
        core_out = res.results[core]["out"]    # [BPC, 128(v), 1024(q)]
        core_sums = res.results[core]["sums"]  # [BPC, 1, 1024(q)]
        for slot in range(BPC):
            bidx = int(perm[slot, core])
            out[bidx] = (core_out[slot] / core_sums[slot]).T
    return out


# revision 21
# speedup vs baseline: 1.2146x; 1.2146x over previous
"""Masked dot-product attention on 8 Trainium2 NeuronCores (Bass/Tile).

Problem: queries/keys/values [32, 1024, 128] f32, valid_lens [32] i32.
  out = softmax(mask(Q K^T / sqrt(128))) V        (key-padding prefix mask)

Strategy (batch-parallel, 4 batches per core, one SPMD program):
  * Host pre-transposes Q and K per batch to [D=128, 1024] so the
    contraction dim D sits on SBUF partitions; no on-device transposes.
  * Scores are computed transposed: S^T[k, q] = (K^T chunk).T @ Q^T with k
    in chunks of 128 partitions.
  * The prefix key mask is per-PARTITION in this layout, so it folds into
    the exp for free: ACT computes exp(S^T * 1/sqrt(D) + bias) with
    bias[k] in {0, -1e6}; masked rows become exactly 0.
  * out^T[v, q] += V_chunk-as-lhsT @ expS^T accumulates in PSUM across
    k chunks (V is loaded chunk-major, no transpose needed).
  * denominator[q] = ones-column matmuls on the same expS^T chunks,
    accumulated in PSUM (exact: multiply by 1.0).
  * out^T and sums are DMA'd back; the host divides and transposes
    while gathering (0.003% of the FLOPs).
  * float32r everywhere on the PE: 1 cycle/row instead of fp32's 4.

Static masked-chunk skipping: batch b only needs ceil(valid_lens[b]/128)
key chunks; the rest contribute exactly 0. Batches are assigned to the 4
per-core slots by descending need (sorted, slot-major), so slot j's
compile-time chunk count is max over its 8 batches. The SPMD program is
specialized to that profile at kernel build time.

The chunk loop is software-pipelined: chunk c+1's score matmuls are
emitted before chunk c's AV/sums matmuls so the PE produces the next
exp's input first and ACT never starves.
"""

import math
import os as _os

import numpy as np

import concourse.bacc as bacc
import concourse.bass as bass
import concourse.mybir as mybir
import concourse.tile as tile
from concourse.bass_utils import run_bass_kernel_spmd

B, Q, K, D = 32, 1024, 1024, 128
N_CORES = 8
BPC = B // N_CORES  # batches per core
PART = 128          # partition size / key chunk size
NCHUNK = K // PART
MASK_BIAS = -1.0e6
INV_SQRT_D = 1.0 / math.sqrt(D)
F32 = mybir.dt.float32
F32R = mybir.dt.float32r

_NC_CACHE: dict = {}


def build_nc(profile: tuple) -> bass.Bass:
    """Build the SPMD Bass program for a per-slot chunk-count profile."""
    nc = bacc.Bacc()
    qt = nc.declare_dram_parameter("qt", [BPC, PART, Q], F32R, isOutput=False)
    kt = nc.declare_dram_parameter("kt", [BPC, PART, K], F32R, isOutput=False)
    vp = nc.declare_dram_parameter("vp", [BPC, PART, K], F32R, isOutput=False)
    mb = nc.declare_dram_parameter("mb", [PART, BPC * NCHUNK], F32, isOutput=False)
    cst = nc.declare_dram_parameter("cst", [PART, PART], F32R, isOutput=False)
    out = nc.declare_dram_parameter("out", [BPC, PART, Q], F32, isOutput=True)
    sums_out = nc.declare_dram_parameter("sums", [BPC, 1, Q], F32, isOutput=True)

    with tile.TileContext(nc) as tc:
        with (
            tc.tile_pool(name="io", bufs=2) as io,
            tc.tile_pool(name="probs", bufs=8) as probs,
            tc.tile_pool(name="consts", bufs=1) as consts,
            tc.tile_pool(name="ps_s", bufs=2, space="PSUM") as ps_s,
            tc.tile_pool(name="ps_acc", bufs=1, space="PSUM") as ps_acc,
        ):
            # Startup-ordered loads: batch 0's operands first (SP HWDGE ring
            # is FIFO), then the small consts, then the rest.
            ins_sb = []
            for b in range(BPC):
                cap = profile[b]
                kcols = cap * PART
                qt_sb = io.tile([PART, Q], F32R, tag="qt", name=f"qt{b}")
                kt_sb = io.tile([PART, kcols], F32R, tag="kt", name=f"kt{b}")
                vp_sb = io.tile([PART, kcols], F32R, tag="vp", name=f"vp{b}")
                ins_sb.append((qt_sb, kt_sb, vp_sb))
                nc.sync.dma_start(out=kt_sb, in_=kt[b][:, :kcols])
                nc.sync.dma_start(out=qt_sb, in_=qt[b])
                if b == 0:
                    mb_sb = consts.tile([PART, BPC * NCHUNK], F32)
                    nc.sync.dma_start(out=mb_sb, in_=mb[:, :])
                    cst_sb = consts.tile([PART, PART], F32R)
                    nc.sync.dma_start(out=cst_sb, in_=cst[:, :])
                    ones_col = cst_sb[:, 0:1]
                nc.sync.dma_start(out=vp_sb, in_=vp[b][:, :kcols])

            for b in range(BPC):
                cap = profile[b]
                qt_sb, kt_sb, vp_sb = ins_sb[b]

                out_ps = ps_acc.tile([PART, Q], F32, tag="out")
                sums_ps = ps_acc.tile([1, Q], F32, tag="sums")

                def s_mms(c):
                    s_ps = ps_s.tile([PART, Q], F32, tag="s", name=f"s_b{b}c{c}")
                    kw = kt_sb[:, c * PART:(c + 1) * PART]
                    for h in range(2):
                        nc.tensor.matmul(
                            s_ps[:, h * 512:(h + 1) * 512],
                            kw,
                            qt_sb[:, h * 512:(h + 1) * 512],
                            start=True,
                            stop=True,
                        )
                    return s_ps

                s_tiles = {0: s_mms(0)}
                if cap > 1:
                    s_tiles[1] = s_mms(1)
                for c in range(cap):
                    p_sb = probs.tile([PART, Q], F32R, tag="p")
                    nc.scalar.activation(
                        p_sb,
                        s_tiles.pop(c),
                        mybir.ActivationFunctionType.Exp,
                        bias=mb_sb[:, b * NCHUNK + c:b * NCHUNK + c + 1],
                        scale=INV_SQRT_D,
                    )
                    if c + 2 < cap:
                        s_tiles[c + 2] = s_mms(c + 2)
                    vw = vp_sb[:, c * PART:(c + 1) * PART]
                    first, last = c == 0, c == cap - 1
                    for h in range(2):
                        nc.tensor.matmul(
                            out_ps[:, h * 512:(h + 1) * 512],
                            vw,
                            p_sb[:, h * 512:(h + 1) * 512],
                            start=first,
                            stop=last,
                        )
                    for h in range(2):
                        nc.tensor.matmul(
                            sums_ps[:, h * 512:(h + 1) * 512],
                            ones_col[:, :],
                            p_sb[:, h * 512:(h + 1) * 512],
                            start=first,
                            stop=last,
                        )

                # Epilogue: PSUM -> SBUF on DVE only (ACT queue stays pure
                # exp), then DMA out; host divides.
                outn = io.tile([PART, Q], F32, tag="outn")
                nc.vector.tensor_copy(outn[:, 0:512], out_ps[:, 0:512])
                nc.vector.tensor_copy(outn[:, 512:1024], out_ps[:, 512:1024])
                nc.sync.dma_start(out=out[b], in_=outn)
                sums_sb = probs.tile([1, Q], F32, tag="sums_sb")
                nc.vector.tensor_copy(sums_sb, sums_ps)
                nc.sync.dma_start(out=sums_out[b], in_=sums_sb)

    nc.compile()
    return nc


def plan(valid_lens: np.ndarray):
    """Assign batches to (core, slot) and derive the chunk-count profile.

    Sorting by descending need and slicing slot-major minimizes the sum of
    per-slot maxima, which is the per-core static work.
    """
    need = np.minimum((valid_lens.astype(np.int64) + PART - 1) // PART, NCHUNK)
    need = np.maximum(need, 1)
    order = np.argsort(-need, kind="stable")
    perm = order.reshape(BPC, N_CORES)  # perm[slot, core] = batch index
    profile = tuple(int(need[perm[s]].max()) for s in range(BPC))
    return perm, profile


def kernel(queries, keys, values, valid_lens):
    q = np.ascontiguousarray(np.asarray(queries, dtype=np.float32))
    k = np.ascontiguousarray(np.asarray(keys, dtype=np.float32))
    v = np.ascontiguousarray(np.asarray(values, dtype=np.float32))
    lens = np.asarray(valid_lens).astype(np.int64).reshape(B)

    perm, profile = plan(lens)

    if profile not in _NC_CACHE:
        _NC_CACHE[profile] = build_nc(profile)
    nc = _NC_CACHE[profile]

    # Vectorized host layout prep: obi[core, slot] = batch index.
    obi = perm.T  # [N_CORES, BPC]
    qt_all = np.ascontiguousarray(q[obi].transpose(0, 1, 3, 2))  # [8,4,128,1024]
    kt_all = np.ascontiguousarray(k[obi].transpose(0, 1, 3, 2))
    # v chunk-major: vp[p, c*128 + d] = v[c*128 + p, d]
    vp_all = np.ascontiguousarray(
        v[obi]
        .reshape(N_CORES, BPC, NCHUNK, PART, D)
        .transpose(0, 1, 3, 2, 4)
        .reshape(N_CORES, BPC, PART, K)
    )
    # bias[p, slot*8 + c] = 0 if (c*128+p) < L else -1e6
    valid = np.arange(K)[None, None, :] < lens[obi][:, :, None]  # [8,4,1024]
    mb_all = np.where(
        valid.reshape(N_CORES, BPC, NCHUNK, PART).transpose(0, 2, 3, 1), 0.0, MASK_BIAS
    ).astype(np.float32)  # [8, NCHUNK, PART, BPC] -> need [8, PART, BPC*NCHUNK]
    mb_all = np.ascontiguousarray(
        mb_all.transpose(0, 2, 3, 1).reshape(N_CORES, PART, BPC * NCHUNK)
    )
    ones = np.ones((PART, PART), np.float32)

    in_maps = [
        {
            "qt": qt_all[core],
            "kt": kt_all[core],
            "vp": vp_all[core],
            "mb": mb_all[core],
            "cst": ones,
        }
        for core in range(N_CORES)
    ]

    res = run_bass_kernel_spmd(nc, in_maps, list(range(N_CORES)))

    out = np.empty((B, Q, D), np.float32)
    for core in range(N_CORES):
        core_out = res.results[core]["out"]    # [BPC, 128(v), 1024(q)]
        core_sums = res.results[core]["sums"]  # [BPC, 1, 1024(q)]
        for slot in range(BPC):
            bidx = int(perm[slot, core])
            out[bidx] = (core_out[slot] / core_sums[slot]).T
    return out


# revision 22
# speedup vs baseline: 1.2233x; 1.0072x over previous
"""Masked dot-product attention on 8 Trainium2 NeuronCores (Bass/Tile).

Problem: queries/keys/values [32, 1024, 128] f32, valid_lens [32] i32.
  out = softmax(mask(Q K^T / sqrt(128))) V        (key-padding prefix mask)

Strategy (batch-parallel, 4 batches per core, one SPMD program):
  * Host pre-transposes Q and K per batch to [D=128, 1024] so the
    contraction dim D sits on SBUF partitions; no on-device transposes.
  * Scores are computed transposed: S^T[k, q] = (K^T chunk).T @ Q^T with k
    in chunks of 128 partitions.
  * The prefix key mask is per-PARTITION in this layout, so it folds into
    the exp for free: ACT computes exp(S^T * 1/sqrt(D) + bias) with
    bias[k] in {0, -1e6}; masked rows become exactly 0.
  * out^T[v, q] += V_chunk-as-lhsT @ expS^T accumulates in PSUM across
    k chunks (V is loaded chunk-major, no transpose needed).
  * denominator[q] = ones-column matmuls on the same expS^T chunks,
    accumulated in PSUM (exact: multiply by 1.0).
  * out^T and sums are DMA'd back; the host divides and transposes
    while gathering (0.003% of the FLOPs).
  * float32r everywhere on the PE: 1 cycle/row instead of fp32's 4.

Static masked-chunk skipping: batch b only needs ceil(valid_lens[b]/128)
key chunks; the rest contribute exactly 0. Batches are assigned to the 4
per-core slots by descending need (sorted, slot-major), so slot j's
compile-time chunk count is max over its 8 batches. The SPMD program is
specialized to that profile at kernel build time.

The chunk loop is software-pipelined: chunk c+1's score matmuls are
emitted before chunk c's AV/sums matmuls so the PE produces the next
exp's input first and ACT never starves.
"""

import math
import os as _os

import numpy as np

import concourse.bacc as bacc
import concourse.bass as bass
import concourse.mybir as mybir
import concourse.tile as tile
from concourse.bass_utils import run_bass_kernel_spmd

B, Q, K, D = 32, 1024, 1024, 128
N_CORES = 8
BPC = B // N_CORES  # batches per core
PART = 128          # partition size / key chunk size
NCHUNK = K // PART
MASK_BIAS = -1.0e6
INV_SQRT_D = 1.0 / math.sqrt(D)
F32 = mybir.dt.float32
F32R = mybir.dt.float32r

_NC_CACHE: dict = {}


def build_nc(profile: tuple) -> bass.Bass:
    """Build the SPMD Bass program for a per-slot chunk-count profile."""
    nc = bacc.Bacc()
    qt = nc.declare_dram_parameter("qt", [BPC, PART, Q], F32R, isOutput=False)
    kt = nc.declare_dram_parameter("kt", [BPC, PART, K], F32R, isOutput=False)
    vp = nc.declare_dram_parameter("vp", [BPC, PART, K], F32R, isOutput=False)
    mb = nc.declare_dram_parameter("mb", [PART, BPC * NCHUNK], F32, isOutput=False)
    cst = nc.declare_dram_parameter("cst", [PART, PART], F32R, isOutput=False)
    out = nc.declare_dram_parameter("out", [BPC, PART, Q], F32, isOutput=True)
    sums_out = nc.declare_dram_parameter("sums", [BPC, 1, Q], F32, isOutput=True)

    with tile.TileContext(nc) as tc:
        with (
            tc.tile_pool(name="io", bufs=2) as io,
            tc.tile_pool(name="probs", bufs=8) as probs,
            tc.tile_pool(name="consts", bufs=1) as consts,
            tc.tile_pool(name="ps_s", bufs=2, space="PSUM") as ps_s,
            tc.tile_pool(name="ps_acc", bufs=1, space="PSUM") as ps_acc,
        ):
            # Startup-ordered loads: batch 0's operands first (SP HWDGE ring
            # is FIFO), then the small consts, then the rest.
            ins_sb = []
            for b in range(BPC):
                cap = profile[b]
                kcols = cap * PART
                qt_sb = io.tile([PART, Q], F32R, tag="qt", name=f"qt{b}")
                kt_sb = io.tile([PART, kcols], F32R, tag="kt", name=f"kt{b}")
                vp_sb = io.tile([PART, kcols], F32R, tag="vp", name=f"vp{b}")
                ins_sb.append((qt_sb, kt_sb, vp_sb))
                nc.sync.dma_start(out=kt_sb, in_=kt[b][:, :kcols])
                nc.sync.dma_start(out=qt_sb, in_=qt[b])
                if b == 0:
                    mb_sb = consts.tile([PART, BPC * NCHUNK], F32)
                    nc.sync.dma_start(out=mb_sb, in_=mb[:, :])
                    cst_sb = consts.tile([PART, PART], F32R)
                    nc.sync.dma_start(out=cst_sb, in_=cst[:, :])
                    ones_col = cst_sb[:, 0:1]
                nc.sync.dma_start(out=vp_sb, in_=vp[b][:, :kcols])

            # Flat chunk stream across batches with 2-deep score lookahead:
            # the in-order PE queue must see the next chunks' score matmuls
            # BEFORE a batch-boundary AV matmul that may stall on the PSUM
            # accumulator release.
            stream = [(b, c) for b in range(BPC) for c in range(profile[b])]

            def s_mms(b, c):
                qt_sb, kt_sb, _ = ins_sb[b]
                s_ps = ps_s.tile([PART, Q], F32, tag="s", name=f"s_b{b}c{c}")
                kw = kt_sb[:, c * PART:(c + 1) * PART]
                for h in range(2):
                    nc.tensor.matmul(
                        s_ps[:, h * 512:(h + 1) * 512],
                        kw,
                        qt_sb[:, h * 512:(h + 1) * 512],
                        start=True,
                        stop=True,
                    )
                return s_ps

            s_tiles = {}
            for j in range(min(2, len(stream))):
                s_tiles[stream[j]] = s_mms(*stream[j])
            acc = {}
            for i, (b, c) in enumerate(stream):
                cap = profile[b]
                if c == 0:
                    out_ps = ps_acc.tile(
                        [PART, Q], F32, tag="out", name=f"out_b{b}"
                    )
                    sums_ps = ps_acc.tile(
                        [1, Q], F32, tag="sums", name=f"sums_b{b}"
                    )
                    acc[b] = (out_ps, sums_ps)
                out_ps, sums_ps = acc[b]
                p_sb = probs.tile([PART, Q], F32R, tag="p")
                nc.scalar.activation(
                    p_sb,
                    s_tiles.pop((b, c)),
                    mybir.ActivationFunctionType.Exp,
                    bias=mb_sb[:, b * NCHUNK + c:b * NCHUNK + c + 1],
                    scale=INV_SQRT_D,
                )
                if i + 2 < len(stream):
                    s_tiles[stream[i + 2]] = s_mms(*stream[i + 2])
                vw = ins_sb[b][2][:, c * PART:(c + 1) * PART]
                first, last = c == 0, c == cap - 1
                for h in range(2):
                    nc.tensor.matmul(
                        out_ps[:, h * 512:(h + 1) * 512],
                        vw,
                        p_sb[:, h * 512:(h + 1) * 512],
                        start=first,
                        stop=last,
                    )
                for h in range(2):
                    nc.tensor.matmul(
                        sums_ps[:, h * 512:(h + 1) * 512],
                        ones_col[:, :],
                        p_sb[:, h * 512:(h + 1) * 512],
                        start=first,
                        stop=last,
                    )
                if last:
                    # Epilogue: PSUM -> SBUF split across ACT and DVE so the
                    # accumulator releases quickly, then DMA; host divides.
                    outn = io.tile([PART, Q], F32, tag="outn", name=f"outn{b}")
                    nc.scalar.copy(outn[:, 0:512], out_ps[:, 0:512])
                    nc.vector.tensor_copy(outn[:, 512:1024], out_ps[:, 512:1024])
                    nc.sync.dma_start(out=out[b], in_=outn)
                    sums_sb = probs.tile(
                        [1, Q], F32, tag="sums_sb", name=f"sums_sb{b}"
                    )
                    nc.vector.tensor_copy(sums_sb, sums_ps)
                    nc.sync.dma_start(out=sums_out[b], in_=sums_sb)

    nc.compile()
    return nc


def plan(valid_lens: np.ndarray):
    """Assign batches to (core, slot) and derive the chunk-count profile.

    Sorting by descending need and slicing slot-major minimizes the sum of
    per-slot maxima, which is the per-core static work.
    """
    need = np.minimum((valid_lens.astype(np.int64) + PART - 1) // PART, NCHUNK)
    need = np.maximum(need, 1)
    order = np.argsort(-need, kind="stable")
    perm = order.reshape(BPC, N_CORES)  # perm[slot, core] = batch index
    profile = tuple(int(need[perm[s]].max()) for s in range(BPC))
    return perm, profile


def kernel(queries, keys, values, valid_lens):
    q = np.ascontiguousarray(np.asarray(queries, dtype=np.float32))
    k = np.ascontiguousarray(np.asarray(keys, dtype=np.float32))
    v = np.ascontiguousarray(np.asarray(values, dtype=np.float32))
    lens = np.asarray(valid_lens).astype(np.int64).reshape(B)

    perm, profile = plan(lens)

    if profile not in _NC_CACHE:
        _NC_CACHE[profile] = build_nc(profile)
    nc = _NC_CACHE[profile]

    # Vectorized host layout prep: obi[core, slot] = batch index.
    obi = perm.T  # [N_CORES, BPC]
    qt_all = np.ascontiguousarray(q[obi].transpose(0, 1, 3, 2))  # [8,4,128,1024]
    kt_all = np.ascontiguousarray(k[obi].transpose(0, 1, 3, 2))
    # v chunk-major: vp[p, c*128 + d] = v[c*128 + p, d]
    vp_all = np.ascontiguousarray(
        v[obi]
        .reshape(N_CORES, BPC, NCHUNK, PART, D)
        .transpose(0, 1, 3, 2, 4)
        .reshape(N_CORES, BPC, PART, K)
    )
    # bias[p, slot*8 + c] = 0 if (c*128+p) < L else -1e6
    valid = np.arange(K)[None, None, :] < lens[obi][:, :, None]  # [8,4,1024]
    mb_all = np.where(
        valid.reshape(N_CORES, BPC, NCHUNK, PART).transpose(0, 2, 3, 1), 0.0, MASK_BIAS
    ).astype(np.float32)  # [8, NCHUNK, PART, BPC] -> need [8, PART, BPC*NCHUNK]
    mb_all = np.ascontiguousarray(
        mb_all.transpose(0, 2, 3, 1).reshape(N_CORES, PART, BPC * NCHUNK)
    )
    ones = np.ones((PART, PART), np.float32)

    in_maps = [
        {
            "qt": qt_all[core],
            "kt": kt_all[core],
            "vp": vp_all[core],
            "mb": mb_all[core],
            "cst": ones,
        }
        for core in range(N_CORES)
    ]

    res = run_bass_kernel_spmd(nc, in_maps, list(range(N_CORES)))

    out = np.empty((B, Q, D), np.float32)
    for core in range(N_CORES):
        core_out = res.results[core]["out"]    # [BPC, 128(v), 1024(q)]
        core_sums = res.results[core]["sums"]  # [BPC, 1, 1024(q)]
        for slot in range(BPC):
            bidx = int(perm[slot, core])
            out[bidx] = (core_out[slot] / core_sums[slot]).T
    return out
